# revision 13
# baseline (speedup 1.0000x reference)
"""Trainium2 Bass kernel for the cross-attention module (nn_CIM_34677565948716).

Sharding: 8 cores = 4 batches x 2 query-halves. Each core computes the full
attention for its (batch, 2048-query slice): k/v from the full h[b] (duplicated
across the 2 cores of a batch -- cheap), q/out for its query half only.

Schedule (v2): the kernel is ACT-bound -- exp over 16.8M scores/core is a
~135us floor at 1 elem/lane/cycle -- so everything else is arranged to fit
under the exp curtain:
  - all host->device tensors are laid out partition-major on the host so
    every DMA is one fat contiguous descriptor per partition (v1 lost 26us
    to rearrange-shattered descriptors before the first matmul)
  - convs (h_al, k, vT, q, out-conv) are f32r matmuls at tile_position (0,0)
    with BN folded into the weights host-side; v-bias is a host-replicated
    [128,C] tile added by DVE; out-conv bias+residual ride one DVE
    scalar_tensor_tensor -- no identity/bias matmuls on PE
  - attention per (nqb, head-pair): S^T row-packed 2 heads/span into a
    3-deep [128,1024] PSUM pipeline; ACT exp (PSUM->SBUF bf16, scale
    folded) runs back-to-back; AV + ones-denominator matmuls col-packed
    4/span accumulate over the 8 key blocks
  - normalize: denom rows -> DVE copy -> reciprocal_approx_fast ->
    gpsimd.partition_broadcast (base-0 dest only!) -> DVE mul

HW quirks honored here (found empirically, CoreSim disagrees):
  - reciprocal_approx_fast with PSUM source returns garbage -> copy first
  - partition_broadcast with dest base partition != 0 is a silent no-op
  - f32r matmuls only legal at tile_position (0,0); memset can't write f32r
  - HAM holds the PE at K=4/8 (1.2 GHz) through the whole attention phase
    (per-mb bursts + stalls never look "busy" to it) -- so PE budgets below
    assume 1.2 GHz; the exp curtain still dominates
"""

import numpy as np

import concourse.bacc as bacc
import concourse.bass as bass  # noqa: F401
import concourse.mybir as mybir
import concourse.tile as tile
from concourse.bass_utils import run_bass_kernel_spmd

HEADS = 8
EPS = 1e-5
B, C, H, W = 4, 256, 64, 64
CH, HH, WH = 512, 32, 32
N = H * W          # 4096 query positions per batch
M = HH * WH        # 1024 key positions
NCORES = 8
NLOC = N // 2      # 2048 queries per core
DH = C // HEADS    # 32
SCALE = float(DH) ** -0.5
FP32 = mybir.dt.float32
F32R = mybir.dt.float32r
BF16 = mybir.dt.bfloat16
EXP = mybir.ActivationFunctionType.Exp
ADD = mybir.AluOpType.add

_PROGRAM = None


def _build_program():
    nc = bacc.Bacc("TRN2", target_bir_lowering=False, debug=False)

    f_in = nc.dram_tensor("f_loc", [128, 2, NLOC], F32R, kind="ExternalInput")
    h_in = nc.dram_tensor("h_loc", [128, 4, M], F32R, kind="ExternalInput")
    wht = nc.dram_tensor("wht_t", [128, 4, C], F32R, kind="ExternalInput")
    wq = nc.dram_tensor("wq_t", [128, 2, C], F32R, kind="ExternalInput")
    wk = nc.dram_tensor("wk_t", [128, 2, C], F32R, kind="ExternalInput")
    wv = nc.dram_tensor("wv_t", [128, 2, C], F32R, kind="ExternalInput")
    wo = nc.dram_tensor("wo_t", [128, 2, C], F32R, kind="ExternalInput")
    bht = nc.dram_tensor("b_ht", [128, 2], FP32, kind="ExternalInput")
    bq = nc.dram_tensor("b_q", [128, 2], FP32, kind="ExternalInput")
    bk = nc.dram_tensor("b_k", [128, 2], FP32, kind="ExternalInput")
    bo = nc.dram_tensor("b_o", [128, 2], FP32, kind="ExternalInput")
    bvbc = nc.dram_tensor("bv_bc", [128, C], BF16, kind="ExternalInput")
    out_d = nc.dram_tensor("out_loc", [128, 2, NLOC], FP32, kind="ExternalOutput")

    with tile.TileContext(nc) as tc:
        with tc.tile_pool(name="const", bufs=1) as cp:
            # ---- static loads: contiguous partition-major, k/v deps first ----
            # Fat descriptors: DMA cost is ~68ns/descriptor regardless of
            # size, so ship whole 16KB partition rows and parallelize over
            # rings by splitting the partition dim 4 ways.
            def pdma(dst, src):
                for pg_ in range(4):
                    nc.sync.dma_start(dst[32 * pg_:32 * (pg_ + 1)],
                                      src[32 * pg_:32 * (pg_ + 1)])

            sb_h = cp.tile([128, 4, M], F32R)
            pdma(sb_h, h_in[:])
            sb_wht = cp.tile([128, 4, C], F32R)
            pdma(sb_wht, wht[:])
            sb_wk = cp.tile([128, 2, C], F32R)
            pdma(sb_wk, wk[:])
            sb_wv = cp.tile([128, 2, C], F32R)
            pdma(sb_wv, wv[:])
            sb_bht = cp.tile([128, 2], FP32)
            nc.sync.dma_start(sb_bht[:], bht[:])
            sb_bk = cp.tile([128, 2], FP32)
            nc.sync.dma_start(sb_bk[:], bk[:])
            sb_bvbc = cp.tile([128, C], BF16)
            pdma(sb_bvbc, bvbc[:])
            sb_f = cp.tile([128, 2, NLOC], F32R)
            pdma(sb_f, f_in[:])
            sb_wq = cp.tile([128, 2, C], F32R)
            pdma(sb_wq, wq[:])
            sb_bq = cp.tile([128, 2], FP32)
            nc.sync.dma_start(sb_bq[:], bq[:])
            sb_wo = cp.tile([128, 2, C], F32R)
            pdma(sb_wo, wo[:])
            sb_bo = cp.tile([128, 2], FP32)
            nc.sync.dma_start(sb_bo[:], bo[:])

            ones_col = cp.tile([128, 1], BF16)
            nc.vector.memset(ones_col[:], 1.0)

            # warm the exp table set early (overlaps with DMAs/convs)
            warm = cp.tile([1, 2], FP32)
            nc.scalar.activation(warm[:], sb_bht[0:1, :], EXP)

            sb_hal = cp.tile([128, 2, M], F32R)
            sb_k = cp.tile([128, 2, M], BF16)
            sb_vT = cp.tile([128, 8, C], BF16)
            sb_q = cp.tile([128, 2, NLOC], BF16)
            sb_attn = cp.tile([128, 2, NLOC], F32R)

            # ---- conv phase (f32r matmuls, full array) ----
            with tc.tile_pool(name="cps", bufs=4, space="PSUM") as cps:
                # h_al = wht^T . h + b_ht   (256, 1024); fc-major so the
                # k-conv for fc=0 can start after two groups, not three
                for fc in range(2):
                    for ob in range(2):
                        ps = cps.tile([128, 512], FP32, tag="cps")
                        for kb in range(4):
                            nc.tensor.matmul(
                                ps[:],
                                sb_wht[:, kb, 128 * ob:128 * (ob + 1)],
                                sb_h[:, kb, 512 * fc:512 * (fc + 1)],
                                start=(kb == 0), stop=(kb == 3),
                            )
                        nc.vector.tensor_scalar_add(
                            sb_hal[:, ob, 512 * fc:512 * (fc + 1)], ps[:],
                            sb_bht[:, ob:ob + 1],
                        )
                # k = WK . h_al + bK  -> bf16  (256, 1024)
                for fc in range(2):
                    for ob in range(2):
                        ps = cps.tile([128, 512], FP32, tag="cps")
                        for kb in range(2):
                            nc.tensor.matmul(
                                ps[:],
                                sb_wk[:, kb, 128 * ob:128 * (ob + 1)],
                                sb_hal[:, kb, 512 * fc:512 * (fc + 1)],
                                start=(kb == 0), stop=(kb == 1),
                            )
                        nc.vector.tensor_scalar_add(
                            sb_k[:, ob, 512 * fc:512 * (fc + 1)], ps[:],
                            sb_bk[:, ob:ob + 1],
                        )
                # q block 0 first so attention nqb=0 can start early,
                # then vT (needed by the AV accumulation), then q blocks 1-3
                def q_block(fc):
                    for ob in range(2):
                        ps = cps.tile([128, 512], FP32, tag="cps")
                        for kb in range(2):
                            nc.tensor.matmul(
                                ps[:],
                                sb_wq[:, kb, 128 * ob:128 * (ob + 1)],
                                sb_f[:, kb, 512 * fc:512 * (fc + 1)],
                                start=(kb == 0), stop=(kb == 1),
                            )
                        nc.vector.tensor_scalar_add(
                            sb_q[:, ob, 512 * fc:512 * (fc + 1)], ps[:],
                            sb_bq[:, ob:ob + 1],
                        )

                q_block(0)
                # vT[m, c] = h_al^T . WV^T + bV -> bf16  (1024, 256)
                for mb in range(8):
                    ps = cps.tile([128, 512], FP32, tag="cps")
                    for kb in range(2):
                        nc.tensor.matmul(
                            ps[:, :C],
                            sb_hal[:, kb, 128 * mb:128 * (mb + 1)],
                            sb_wv[:, kb, :],
                            start=(kb == 0), stop=(kb == 1),
                        )
                    nc.vector.tensor_add(
                        out=sb_vT[:, mb, :], in0=ps[:, :C], in1=sb_bvbc[:],
                    )
                for fc in range(1, 4):
                    q_block(fc)

            # ---- attention + output conv (bf16 matmuls, packed) ----
            with (
                tc.tile_pool(name="spool", bufs=2, space="PSUM") as sp,
                tc.tile_pool(name="opool", bufs=2, space="PSUM") as op_,
                tc.tile_pool(name="ocp", bufs=2, space="PSUM") as ocp,
                tc.tile_pool(name="ppool", bufs=3) as pp,
                tc.tile_pool(name="npool", bufs=3) as npo,
            ):
                for nqb in range(4):
                    nq0 = 512 * nqb
                    for pg in range(4):
                        hg = pg // 2
                        out_ps = op_.tile([128, 512], FP32, tag="out")
                        for mb in range(8):
                            s_ps = sp.tile([128, 1024], FP32, tag="s")
                            for j in range(2):
                                jj = (2 * pg + j) % 4
                                nc.tensor.matmul(
                                    s_ps[:, 512 * j:512 * (j + 1)],
                                    sb_k[32 * jj:32 * jj + 32, hg,
                                         128 * mb:128 * (mb + 1)],
                                    sb_q[32 * jj:32 * jj + 32, hg,
                                         nq0:nq0 + 512],
                                    start=True, stop=True,
                                    tile_position=(32 * jj, 0),
                                )
                            p_sb = pp.tile([128, 1024], BF16, tag="p")
                            nc.scalar.activation(p_sb[:], s_ps[:], EXP,
                                                 scale=SCALE)
                            for j in range(2):
                                head = 2 * pg + j
                                nc.tensor.matmul(
                                    out_ps[32 * j:32 * j + 32, :],
                                    sb_vT[:, mb, 32 * head:32 * head + 32],
                                    p_sb[:, 512 * j:512 * (j + 1)],
                                    start=(mb == 0), stop=(mb == 7),
                                    tile_position=(0, 32 * j),
                                    skip_group_check=True,
                                )
                                nc.tensor.matmul(
                                    out_ps[64 + 32 * j:64 + 32 * j + 1, :],
                                    ones_col[:],
                                    p_sb[:, 512 * j:512 * (j + 1)],
                                    start=(mb == 0), stop=(mb == 7),
                                    tile_position=(0, 64 + 32 * j),
                                    skip_group_check=True,
                                )
                        # normalize: rows 0-63 = AV pair, rows 64/96 denoms
                        den0 = npo.tile([1, 512], FP32, tag="den0")
                        den1 = npo.tile([1, 512], FP32, tag="den1")
                        nc.vector.tensor_copy(den0[:], out_ps[64:65, :])
                        nc.vector.tensor_copy(den1[:], out_ps[96:97, :])
                        rec0 = npo.tile([1, 512], FP32, tag="rec0")
                        rec1 = npo.tile([1, 512], FP32, tag="rec1")
                        nc.vector.reciprocal_approx_fast(rec0[:], den0[:])
                        nc.vector.reciprocal_approx_fast(rec1[:], den1[:])
                        bc0 = npo.tile([32, 512], FP32, tag="bc0")
                        bc1 = npo.tile([32, 512], FP32, tag="bc1")
                        nc.gpsimd.partition_broadcast(bc0[:], rec0[:])
                        nc.gpsimd.partition_broadcast(bc1[:], rec1[:])
                        po = 64 * (pg % 2)
                        nc.vector.tensor_mul(
                            out=sb_attn[po:po + 32, hg, nq0:nq0 + 512],
                            in0=out_ps[0:32, :],
                            in1=bc0[:],
                        )
                        nc.vector.tensor_mul(
                            out=sb_attn[po + 32:po + 64, hg, nq0:nq0 + 512],
                            in0=out_ps[32:64, :],
                            in1=bc1[:],
                        )
                    # output conv for this nq block, in its own PSUM pool so
                    # it never stalls the next block's avden pipeline:
                    #   w_out.attn (PE) + b_out + f (one DVE op)
                    for ob in range(2):
                        ps = ocp.tile([128, 512], FP32, tag="oc")
                        for kb in range(2):
                            nc.tensor.matmul(
                                ps[:],
                                sb_wo[:, kb, 128 * ob:128 * (ob + 1)],
                                sb_attn[:, kb, nq0:nq0 + 512],
                                start=(kb == 0), stop=(kb == 1),
                            )
                        fin = npo.tile([128, 512], FP32, tag="fin")
                        nc.vector.scalar_tensor_tensor(
                            out=fin[:], in0=ps[:], scalar=sb_bo[:, ob:ob + 1],
                            in1=sb_f[:, ob, nq0:nq0 + 512],
                            op0=ADD, op1=ADD,
                        )
                        for pg_ in range(4):
                            nc.sync.dma_start(
                                out_d[32 * pg_:32 * (pg_ + 1), ob,
                                      nq0:nq0 + 512],
                                fin[32 * pg_:32 * (pg_ + 1), :])

    nc.compile()
    return nc


def _prep_inputs(inputs):
    """Fold BN into q/k/v weights and build per-core input maps.

    All device tensors are laid out partition-major ([128, ...]) so every
    DMA descriptor is a fat contiguous run per partition.
    """
    g = {k: np.asarray(v, dtype=np.float32) for k, v in inputs.items()}
    a = g["bn_gamma"] / np.sqrt(g["bn_var"] + EPS)        # (3, C)
    c = g["bn_beta"] - g["bn_mean"] * a                   # (3, C)

    WQ = g["w_q"] * a[0][None, :]
    WK = g["w_k"] * a[1][None, :]
    WV = g["w_v"] * a[2][None, :]
    bQ = g["w_q"] @ c[0] + g["b_q"]
    bK = g["w_k"] @ c[1] + g["b_k"]
    bV = g["w_v"] @ c[2] + g["b_v"]

    def pmaj_w(w_t, kb):  # (K, C) -> [128, kb, C]
        return np.ascontiguousarray(
            w_t.reshape(kb, 128, -1).transpose(1, 0, 2))

    def pmaj_b(b):        # (C,) -> [128, 2]
        return np.ascontiguousarray(b.reshape(2, 128).T)

    import ml_dtypes
    shared = {
        "wht_t": pmaj_w(g["w_ht"].T, 4),
        "wq_t": pmaj_w(WQ.T, 2),
        "wk_t": pmaj_w(WK.T, 2),
        "wv_t": pmaj_w(WV.T, 2),
        "wo_t": pmaj_w(g["w_out"].T, 2),
        "b_ht": pmaj_b(g["b_ht"]),
        "b_q": pmaj_b(bQ),
        "b_k": pmaj_b(bK),
        "b_o": pmaj_b(g["b_out"]),
        "bv_bc": np.ascontiguousarray(
            np.tile(bV[None, :], (128, 1)).astype(ml_dtypes.bfloat16)),
    }

    f2 = g["f"].reshape(B, C, N)
    h2 = g["h"].reshape(B, CH, M)
    in_maps = []
    for core in range(NCORES):
        b, hf = core // 2, core % 2
        m = dict(shared)
        floc = f2[b, :, hf * NLOC:(hf + 1) * NLOC]        # (C, NLOC)
        m["f_loc"] = np.ascontiguousarray(
            floc.reshape(2, 128, NLOC).transpose(1, 0, 2))
        m["h_loc"] = np.ascontiguousarray(
            h2[b].reshape(4, 128, M).transpose(1, 0, 2))
        in_maps.append(m)
    return in_maps


def _run(inputs, trace=False):
    global _PROGRAM
    if _PROGRAM is None:
        _PROGRAM = _build_program()
    in_maps = _prep_inputs(inputs)
    try:
        res = run_bass_kernel_spmd(_PROGRAM, in_maps, list(range(NCORES)),
                                   trace=trace)
    except Exception:
        # transient runtime failures have been observed on the tunneled
        # devices; one retry is cheap relative to a failed run
        res = run_bass_kernel_spmd(_PROGRAM, in_maps, list(range(NCORES)),
                                   trace=trace)
    out = np.empty((B, C, N), dtype=np.float32)
    for core in range(NCORES):
        b, hf = core // 2, core % 2
        loc = res.results[core]["out_loc"]                # [128, 2, NLOC]
        out[b, :, hf * NLOC:(hf + 1) * NLOC] = (
            loc.transpose(1, 0, 2).reshape(C, NLOC))
    return out.reshape(B, C, H, W), res


def kernel(**inputs):
    return _run(inputs)[0]


# revision 17
# speedup vs baseline: 1.3550x; 1.3550x over previous
"""Trainium2 Bass kernel for the cross-attention module (nn_CIM_34677565948716).

Sharding: 8 cores = 4 batches x 2 query-halves. Each core computes the full
attention for its (batch, 2048-query slice): k/v from the full h[b] (duplicated
across the 2 cores of a batch -- cheap), q/out for its query half only.

Schedule (v2): the kernel is ACT-bound -- exp over 16.8M scores/core is a
~135us floor at 1 elem/lane/cycle -- so everything else is arranged to fit
under the exp curtain:
  - all host->device tensors are laid out partition-major on the host so
    every DMA is one fat contiguous descriptor per partition (v1 lost 26us
    to rearrange-shattered descriptors before the first matmul)
  - convs (h_al, k, vT, q, out-conv) are f32r matmuls at tile_position (0,0)
    with BN folded into the weights host-side; v-bias is a host-replicated
    [128,C] tile added by DVE; out-conv bias+residual ride one DVE
    scalar_tensor_tensor -- no identity/bias matmuls on PE
  - attention per (nqb, head-pair): S^T row-packed 2 heads/span into a
    3-deep [128,1024] PSUM pipeline; ACT exp (PSUM->SBUF bf16, scale
    folded) runs back-to-back; AV + ones-denominator matmuls col-packed
    4/span accumulate over the 8 key blocks
  - normalize: denom rows -> DVE copy -> reciprocal_approx_fast ->
    gpsimd.partition_broadcast (base-0 dest only!) -> DVE mul

HW quirks honored here (found empirically, CoreSim disagrees):
  - reciprocal_approx_fast with PSUM source returns garbage -> copy first
  - partition_broadcast with dest base partition != 0 is a silent no-op
  - f32r matmuls only legal at tile_position (0,0); memset can't write f32r
  - HAM holds the PE at K=4/8 (1.2 GHz) through the whole attention phase
    (per-mb bursts + stalls never look "busy" to it) -- so PE budgets below
    assume 1.2 GHz; the exp curtain still dominates
"""

import numpy as np

import concourse.bacc as bacc
import concourse.bass as bass  # noqa: F401
import concourse.mybir as mybir
import concourse.tile as tile
from concourse.bass_utils import run_bass_kernel_spmd

HEADS = 8
EPS = 1e-5
B, C, H, W = 4, 256, 64, 64
CH, HH, WH = 512, 32, 32
N = H * W          # 4096 query positions per batch
M = HH * WH        # 1024 key positions
NCORES = 8
NLOC = N // 2      # 2048 queries per core
DH = C // HEADS    # 32
SCALE = float(DH) ** -0.5
FP32 = mybir.dt.float32
F32R = mybir.dt.float32r
BF16 = mybir.dt.bfloat16
EXP = mybir.ActivationFunctionType.Exp
ADD = mybir.AluOpType.add

_PROGRAM = None


def _build_program():
    nc = bacc.Bacc("TRN2", target_bir_lowering=False, debug=False)

    f_in = nc.dram_tensor("f_loc", [128, 2, NLOC], F32R, kind="ExternalInput")
    h_in = nc.dram_tensor("h_loc", [128, 4, M], F32R, kind="ExternalInput")
    wht = nc.dram_tensor("wht_t", [128, 4, C], F32R, kind="ExternalInput")
    wq = nc.dram_tensor("wq_t", [128, 2, C], F32R, kind="ExternalInput")
    wk = nc.dram_tensor("wk_t", [128, 2, C], F32R, kind="ExternalInput")
    wv = nc.dram_tensor("wv_t", [128, 2, C], F32R, kind="ExternalInput")
    wo = nc.dram_tensor("wo_t", [128, 2, C], F32R, kind="ExternalInput")
    bht = nc.dram_tensor("b_ht", [128, 2], FP32, kind="ExternalInput")
    bq = nc.dram_tensor("b_q", [128, 2], FP32, kind="ExternalInput")
    bk = nc.dram_tensor("b_k", [128, 2], FP32, kind="ExternalInput")
    bo = nc.dram_tensor("b_o", [128, 2], FP32, kind="ExternalInput")
    bvbc = nc.dram_tensor("bv_bc", [128, C], BF16, kind="ExternalInput")
    out_d = nc.dram_tensor("out_loc", [128, 2, NLOC], FP32, kind="ExternalOutput")

    with tile.TileContext(nc) as tc:
        with tc.tile_pool(name="const", bufs=1) as cp:
            # ---- static loads: contiguous partition-major, k/v deps first ----
            # Input DMA: HBM-bound (~5.5MB total), so ISSUE ORDER is the
            # lever -- the first 16 dma_starts claim the 16 rings, and the
            # attention-critical wave (h, wht, wk, wv) must own them; f and
            # the q/out weights queue behind on the same rings.
            sb_h = cp.tile([128, 4, M], F32R)
            for kb in range(4):
                for hc in range(2):
                    nc.sync.dma_start(sb_h[:, kb, 512 * hc:512 * (hc + 1)],
                                      h_in[:, kb, 512 * hc:512 * (hc + 1)])
            sb_wht = cp.tile([128, 4, C], F32R)
            for kb in range(4):
                nc.sync.dma_start(sb_wht[:, kb, :], wht[:, kb, :])
            sb_wk = cp.tile([128, 2, C], F32R)
            nc.sync.dma_start(sb_wk[:], wk[:])
            sb_wv = cp.tile([128, 2, C], F32R)
            nc.sync.dma_start(sb_wv[:], wv[:])
            sb_bht = cp.tile([128, 2], FP32)
            nc.sync.dma_start(sb_bht[:], bht[:])
            sb_bk = cp.tile([128, 2], FP32)
            nc.sync.dma_start(sb_bk[:], bk[:])
            sb_bvbc = cp.tile([128, C], BF16)
            nc.sync.dma_start(sb_bvbc[:], bvbc[:])
            sb_f = cp.tile([128, 2, NLOC], F32R)
            for ob in range(2):
                for fc in range(4):
                    nc.sync.dma_start(sb_f[:, ob, 512 * fc:512 * (fc + 1)],
                                      f_in[:, ob, 512 * fc:512 * (fc + 1)])
            sb_wq = cp.tile([128, 2, C], F32R)
            nc.sync.dma_start(sb_wq[:], wq[:])
            sb_bq = cp.tile([128, 2], FP32)
            nc.sync.dma_start(sb_bq[:], bq[:])
            sb_wo = cp.tile([128, 2, C], F32R)
            nc.sync.dma_start(sb_wo[:], wo[:])
            sb_bo = cp.tile([128, 2], FP32)
            nc.sync.dma_start(sb_bo[:], bo[:])

            ones_col = cp.tile([128, 1], BF16)
            nc.vector.memset(ones_col[:], 1.0)

            # warm the exp table set early (overlaps with DMAs/convs)
            warm = cp.tile([1, 2], FP32)
            nc.scalar.activation(warm[:], sb_bht[0:1, :], EXP)

            sb_hal = cp.tile([128, 2, M], F32R)
            sb_k = cp.tile([128, 2, M], BF16)
            sb_vT = cp.tile([128, 8, C], BF16)
            sb_q = cp.tile([128, 2, NLOC], BF16)
            sb_attn = cp.tile([128, 2, NLOC], F32R)

            # ---- conv phase (f32r matmuls, full array) ----
            with tc.tile_pool(name="cps", bufs=4, space="PSUM") as cps:
                # h_al = wht^T . h + b_ht   (256, 1024); fc-major so the
                # k-conv for fc=0 can start after two groups, not three
                for fc in range(2):
                    for ob in range(2):
                        ps = cps.tile([128, 512], FP32, tag="cps")
                        for kb in range(4):
                            nc.tensor.matmul(
                                ps[:],
                                sb_wht[:, kb, 128 * ob:128 * (ob + 1)],
                                sb_h[:, kb, 512 * fc:512 * (fc + 1)],
                                start=(kb == 0), stop=(kb == 3),
                            )
                        nc.vector.tensor_scalar_add(
                            sb_hal[:, ob, 512 * fc:512 * (fc + 1)], ps[:],
                            sb_bht[:, ob:ob + 1],
                        )
                # k = WK . h_al + bK  -> bf16  (256, 1024)
                for fc in range(2):
                    for ob in range(2):
                        ps = cps.tile([128, 512], FP32, tag="cps")
                        for kb in range(2):
                            nc.tensor.matmul(
                                ps[:],
                                sb_wk[:, kb, 128 * ob:128 * (ob + 1)],
                                sb_hal[:, kb, 512 * fc:512 * (fc + 1)],
                                start=(kb == 0), stop=(kb == 1),
                            )
                        nc.vector.tensor_scalar_add(
                            sb_k[:, ob, 512 * fc:512 * (fc + 1)], ps[:],
                            sb_bk[:, ob:ob + 1],
                        )
                # q block 0 first so attention nqb=0 can start early,
                # then vT (needed by the AV accumulation), then q blocks 1-3
                def q_block(fc):
                    for ob in range(2):
                        ps = cps.tile([128, 512], FP32, tag="cps")
                        for kb in range(2):
                            nc.tensor.matmul(
                                ps[:],
                                sb_wq[:, kb, 128 * ob:128 * (ob + 1)],
                                sb_f[:, kb, 512 * fc:512 * (fc + 1)],
                                start=(kb == 0), stop=(kb == 1),
                            )
                        nc.vector.tensor_scalar_add(
                            sb_q[:, ob, 512 * fc:512 * (fc + 1)], ps[:],
                            sb_bq[:, ob:ob + 1],
                        )

                q_block(0)
                # vT[m, c] = h_al^T . WV^T + bV -> bf16  (1024, 256)
                for mb in range(8):
                    ps = cps.tile([128, 512], FP32, tag="cps")
                    for kb in range(2):
                        nc.tensor.matmul(
                            ps[:, :C],
                            sb_hal[:, kb, 128 * mb:128 * (mb + 1)],
                            sb_wv[:, kb, :],
                            start=(kb == 0), stop=(kb == 1),
                        )
                    nc.vector.tensor_add(
                        out=sb_vT[:, mb, :], in0=ps[:, :C], in1=sb_bvbc[:],
                    )
                for fc in range(1, 4):
                    q_block(fc)

            # ---- attention + output conv (bf16 matmuls, packed) ----
            with (
                tc.tile_pool(name="spool", bufs=3, space="PSUM") as sp,
                tc.tile_pool(name="opool", bufs=2, space="PSUM") as op_,
                tc.tile_pool(name="ppool", bufs=3) as pp,
                tc.tile_pool(name="npool", bufs=3) as npo,
            ):
                def emit_oc(nqb_):
                    # output conv for one finished nq block:
                    #   w_out.attn (PE) + b_out + f (one DVE op)
                    nq0_ = 512 * nqb_
                    for ob in range(2):
                        ps = op_.tile([128, 512], FP32, tag="out")
                        for kb in range(2):
                            nc.tensor.matmul(
                                ps[:],
                                sb_wo[:, kb, 128 * ob:128 * (ob + 1)],
                                sb_attn[:, kb, nq0_:nq0_ + 512],
                                start=(kb == 0), stop=(kb == 1),
                            )
                        fin = npo.tile([128, 512], FP32, tag="fin")
                        nc.vector.scalar_tensor_tensor(
                            out=fin[:], in0=ps[:], scalar=sb_bo[:, ob:ob + 1],
                            in1=sb_f[:, ob, nq0_:nq0_ + 512],
                            op0=ADD, op1=ADD,
                        )
                        nc.sync.dma_start(out_d[:, ob, nq0_:nq0_ + 512],
                                          fin[:])

                for nqb in range(4):
                    nq0 = 512 * nqb
                    for pg in range(4):
                        hg = pg // 2
                        out_ps = op_.tile([128, 512], FP32, tag="out")
                        for mb in range(8):
                            s_ps = sp.tile([128, 1024], FP32, tag="s")
                            for j in range(2):
                                jj = (2 * pg + j) % 4
                                nc.tensor.matmul(
                                    s_ps[:, 512 * j:512 * (j + 1)],
                                    sb_k[32 * jj:32 * jj + 32, hg,
                                         128 * mb:128 * (mb + 1)],
                                    sb_q[32 * jj:32 * jj + 32, hg,
                                         nq0:nq0 + 512],
                                    start=True, stop=True,
                                    tile_position=(32 * jj, 0),
                                )
                            p_sb = pp.tile([128, 1024], BF16, tag="p")
                            nc.scalar.activation(p_sb[:], s_ps[:], EXP,
                                                 scale=SCALE)
                            for j in range(2):
                                head = 2 * pg + j
                                nc.tensor.matmul(
                                    out_ps[32 * j:32 * j + 32, :],
                                    sb_vT[:, mb, 32 * head:32 * head + 32],
                                    p_sb[:, 512 * j:512 * (j + 1)],
                                    start=(mb == 0), stop=(mb == 7),
                                    tile_position=(0, 32 * j),
                                    skip_group_check=True,
                                )
                                nc.tensor.matmul(
                                    out_ps[64 + 32 * j:64 + 32 * j + 1, :],
                                    ones_col[:],
                                    p_sb[:, 512 * j:512 * (j + 1)],
                                    start=(mb == 0), stop=(mb == 7),
                                    tile_position=(0, 64 + 32 * j),
                                    skip_group_check=True,
                                )
                        # normalize: rows 0-63 = AV pair, rows 64/96 denoms
                        den0 = npo.tile([1, 512], FP32, tag="den0")
                        den1 = npo.tile([1, 512], FP32, tag="den1")
                        nc.vector.tensor_copy(den0[:], out_ps[64:65, :])
                        nc.vector.tensor_copy(den1[:], out_ps[96:97, :])
                        rec0 = npo.tile([1, 512], FP32, tag="rec0")
                        rec1 = npo.tile([1, 512], FP32, tag="rec1")
                        nc.vector.reciprocal_approx_fast(rec0[:], den0[:])
                        nc.vector.reciprocal_approx_fast(rec1[:], den1[:])
                        bc0 = npo.tile([32, 512], FP32, tag="bc0")
                        bc1 = npo.tile([32, 512], FP32, tag="bc1")
                        nc.gpsimd.partition_broadcast(bc0[:], rec0[:])
                        nc.gpsimd.partition_broadcast(bc1[:], rec1[:])
                        po = 64 * (pg % 2)
                        nc.vector.tensor_mul(
                            out=sb_attn[po:po + 32, hg, nq0:nq0 + 512],
                            in0=out_ps[0:32, :],
                            in1=bc0[:],
                        )
                        nc.vector.tensor_mul(
                            out=sb_attn[po + 32:po + 64, hg, nq0:nq0 + 512],
                            in0=out_ps[32:64, :],
                            in1=bc1[:],
                        )
                        # slot the previous block's out-conv in after pg1:
                        # by now its avden tiles have cycled out, so the
                        # pool interleave costs no stall
                        if pg == 1 and nqb > 0:
                            emit_oc(nqb - 1)
                emit_oc(3)

    nc.compile()
    return nc


def _prep_inputs(inputs):
    """Fold BN into q/k/v weights and build per-core input maps.

    All device tensors are laid out partition-major ([128, ...]) so every
    DMA descriptor is a fat contiguous run per partition.
    """
    g = {k: np.asarray(v, dtype=np.float32) for k, v in inputs.items()}
    a = g["bn_gamma"] / np.sqrt(g["bn_var"] + EPS)        # (3, C)
    c = g["bn_beta"] - g["bn_mean"] * a                   # (3, C)

    WQ = g["w_q"] * a[0][None, :]
    WK = g["w_k"] * a[1][None, :]
    WV = g["w_v"] * a[2][None, :]
    bQ = g["w_q"] @ c[0] + g["b_q"]
    bK = g["w_k"] @ c[1] + g["b_k"]
    bV = g["w_v"] @ c[2] + g["b_v"]

    def pmaj_w(w_t, kb):  # (K, C) -> [128, kb, C]
        return np.ascontiguousarray(
            w_t.reshape(kb, 128, -1).transpose(1, 0, 2))

    def pmaj_b(b):        # (C,) -> [128, 2]
        return np.ascontiguousarray(b.reshape(2, 128).T)

    import ml_dtypes
    shared = {
        "wht_t": pmaj_w(g["w_ht"].T, 4),
        "wq_t": pmaj_w(WQ.T, 2),
        "wk_t": pmaj_w(WK.T, 2),
        "wv_t": pmaj_w(WV.T, 2),
        "wo_t": pmaj_w(g["w_out"].T, 2),
        "b_ht": pmaj_b(g["b_ht"]),
        "b_q": pmaj_b(bQ),
        "b_k": pmaj_b(bK),
        "b_o": pmaj_b(g["b_out"]),
        "bv_bc": np.ascontiguousarray(
            np.tile(bV[None, :], (128, 1)).astype(ml_dtypes.bfloat16)),
    }

    f2 = g["f"].reshape(B, C, N)
    h2 = g["h"].reshape(B, CH, M)
    in_maps = []
    for core in range(NCORES):
        b, hf = core // 2, core % 2
        m = dict(shared)
        floc = f2[b, :, hf * NLOC:(hf + 1) * NLOC]        # (C, NLOC)
        m["f_loc"] = np.ascontiguousarray(
            floc.reshape(2, 128, NLOC).transpose(1, 0, 2))
        m["h_loc"] = np.ascontiguousarray(
            h2[b].reshape(4, 128, M).transpose(1, 0, 2))
        in_maps.append(m)
    return in_maps


def _run(inputs, trace=False):
    global _PROGRAM
    if _PROGRAM is None:
        _PROGRAM = _build_program()
    in_maps = _prep_inputs(inputs)
    try:
        res = run_bass_kernel_spmd(_PROGRAM, in_maps, list(range(NCORES)),
                                   trace=trace)
    except Exception:
        # transient runtime failures have been observed on the tunneled
        # devices; one retry is cheap relative to a failed run
        res = run_bass_kernel_spmd(_PROGRAM, in_maps, list(range(NCORES)),
                                   trace=trace)
    out = np.empty((B, C, N), dtype=np.float32)
    for core in range(NCORES):
        b, hf = core // 2, core % 2
        loc = res.results[core]["out_loc"]                # [128, 2, NLOC]
        out[b, :, hf * NLOC:(hf + 1) * NLOC] = (
            loc.transpose(1, 0, 2).reshape(C, NLOC))
    return out.reshape(B, C, H, W), res


def kernel(**inputs):
    return _run(inputs)[0]
